# revision 29
# baseline (speedup 1.0000x reference)
"""Trainium2 Bass kernel for BondLengthConstraintEnergy.

Contract: kernel(**inputs) takes FULL unsharded inputs (as produced by the
problem's setup_inputs) and returns the FULL output [B, NCH, NRES, n_alt].

Strategy
--------
The input layout produced by setup_inputs is canonical: atom i corresponds to
(b, ch, r, a) = unravel(i) over (32, 8, 8192, 3), so the (b,ch,r,atom)->row
lookup table is exactly arange, every peptide bond (b,ch,r)->(b,ch,r+1) is
present, and the per-residue-type mean/std tables have identical rows.  Under
those conditions (verified on the host each call) the whole computation
collapses to a pure streaming stencil over coords:

  per bond r (residue r, r+1 in the same chain):
    b = C_r - CA_r          (v_cac_c)
    w = N_{r+1} - C_r       (v_cn)
    a = CA_{r+1} - N_{r+1}  (v_nca_n)
    ang1 = angle(w, a), ang2 = angle(b, -w), len = |w|
    lp_i  = min(K_i d_i^2, clip_i);  out = (lp0+lp1+lp2) * f

Angles via theta = pi/2 - arctan(dot / sqrt(q - dot^2)) with q the product
of squared norms; 1/(q-dot^2) comes from a fused custom-DVE op (bitwise-NOT
exponent-flip seed + one Newton step, ~0.2% max err, well inside the loose
gaussian-clip tolerance), sqrt on the scalar engine, and the
scale/bias/square/min/add tail is fused into single custom-DVE ops with all
mean/std/weight-derived constants baked at compile time (the per-core
program is cached keyed on those values).

Sharding: data-parallel over batch, 4 structures per core, no communication.
Each core streams 9.4 MB of coords and writes 1 MB of energies.

If the host-side structure checks fail (inputs are not canonical), we fall
back to a faithful numpy implementation of the reference.
"""

import os
import sys

import numpy as np

for _p in ("/opt/trn_rl_repo",):
    if os.path.isdir(_p) and _p not in sys.path:
        sys.path.insert(0, _p)

# ---------------------------------------------------------------- constants
B, NCH, NRES, APR = 32, 8, 8192, 3
N_ATOMS = B * NCH * NRES * APR
NCORES = 8
B_PER_CORE = B // NCORES
RES_PER_CORE = B_PER_CORE * NCH * NRES          # 262144
ATOMS_PER_CORE = RES_PER_CORE * APR
P = 128                                          # SBUF partitions
RES_PER_PART = RES_PER_CORE // P                 # 2048
EPS = 1e-8
NEG_LOG_EPS = 18.420680743952367                 # -ln(1e-8)
R2D = 180.0 / np.pi
TINY = 1e-30

# NOT-trick reciprocal seed + 1 Newton step constants (see dve_ops.py
# RECIPROCAL_APPROX_FAST; with a single NR pass max rel err ~1.7e-3).
RCP_C1 = -0.23549792
RCP_C2 = 2.0017324

# benign pad residue (N=(0,0,0), CA=(1,0,0), C=(2,0,0)) keeps the one
# out-of-range halo bond finite; its output is overwritten on the host.
_PAD_RESIDUE = np.array([0, 0, 0, 1, 0, 0, 2, 0, 0], dtype=np.float32)

_PROGRAMS = {}


# ------------------------------------------------------------ custom DVE ops
def _ensure_custom_ops():
    """Register the fused DVE ops used by the kernel (idempotent).

    BLC_VRECIP   : out ~= 1/max(in1 - in0^2, s0)       (seed + 1 NR)
    BLC_MULCLAMP : out  = clamp(in0*in1, s1, s0)
    BLC_SQMIN    : out  = min((in0*s0 + s1)^2, imm2)
    BLC_SQMINADD : out  = min((in0*s0 + s1)^2, imm2) + in1
    """
    from concourse import dve_ops as D
    from concourse.dve_spec import (
        AluOp, Bin, Spec, Src0, Src1, C0, C1, C2, lower, maxx, minn, sq,
    )
    from concourse.dve_uop import DveOpSpec

    if "BLC_VRECIP" in D._SUB_OPCODE_FOR_NAME:
        return

    def ref_vrecip(in0, in1, s0, s1, imm2):
        x = np.asarray(in0, np.float32)
        v = np.maximum(np.asarray(in1, np.float32) - x * x,
                       np.float32(s0)).astype(np.float32)
        nx = (~v.view(np.int32)).view(np.float32)
        y0 = (nx * np.float32(s1)).astype(np.float32)
        return (y0 * (np.float32(imm2) - v * y0)).astype(np.float32)

    def ref_mulclamp(in0, in1, s0, s1, imm2):
        t = (np.asarray(in0, np.float32) * np.asarray(in1, np.float32))
        return np.maximum(np.minimum(t, np.float32(s0)),
                          np.float32(s1)).astype(np.float32)

    def ref_sqmin(in0, in1, s0, s1, imm2):
        t = np.asarray(in0, np.float32) * np.float32(s0) + np.float32(s1)
        return np.minimum((t * t).astype(np.float32),
                          np.float32(imm2)).astype(np.float32)

    def ref_sqminadd(in0, in1, s0, s1, imm2):
        t = np.asarray(in0, np.float32) * np.float32(s0) + np.float32(s1)
        return (np.minimum((t * t).astype(np.float32), np.float32(imm2))
                + np.asarray(in1, np.float32)).astype(np.float32)

    def ref_scanmul(in0, in1, s0, s1, imm2):
        a = np.asarray(in0, np.float32)
        b = np.asarray(in1, np.float32)
        p = (a * b).reshape(a.shape[0], -1)
        return np.cumsum(p, axis=-1, dtype=np.float32).reshape(
            a.shape).astype(np.float32)

    from concourse.dve_spec import scan

    v = maxx(Src1 - sq(Src0), C0)
    nx = Bin(AluOp.BITWISE_NOT, v, v)
    y0 = nx * C1
    specs = [
        ("BLC_VRECIP", Spec(body=y0 * (C2 - v * y0), reference=ref_vrecip)),
        ("BLC_MULCLAMP", Spec(body=maxx(minn(Src0 * Src1, C0), C1),
                              reference=ref_mulclamp)),
        ("BLC_SQMIN", Spec(body=minn(sq(Src0 * C0 + C1), C2),
                           reference=ref_sqmin)),
        ("BLC_SQMINADD", Spec(body=minn(sq(Src0 * C0 + C1), C2) + Src1,
                              reference=ref_sqminadd)),
        ("BLC_SCANMUL", Spec(body=scan(AluOp.ADD, Src0 * Src1),
                             reference=ref_scanmul)),
    ]
    from concourse.dve_spec import _has_src1 as has_src1

    for name, spec in specs:
        opcode = D._CUSTOM_DVE_ROW_BASE + len(D.OPS)
        op = D.DveOp(name, spec, False, {})
        assert opcode < 0x20
        for ver in ("v3", "v4"):
            s = DveOpSpec(name=name, opcode=opcode,
                          uops=lower(spec, ver=ver), rd1_en=has_src1(spec))
            op.uops_sha[ver] = s.sha(ver)
        D.OPS.append(op)
        D.CUSTOM_DVE_SPECS[name] = spec
        D._SUB_OPCODE_FOR_NAME[name] = opcode
    return


def _get_ops():
    _ensure_custom_ops()
    from concourse import dve_ops as D
    by = {op.name: op for op in D.OPS}
    return (by["BLC_VRECIP"], by["BLC_MULCLAMP"], by["BLC_SQMIN"],
            by["BLC_SQMINADD"], by["BLC_SCANMUL"])


# ---------------------------------------------------------------- device IR
def _default_params():
    m = np.array([1.33, 121.7, 116.2], np.float64)
    s = np.array([0.02, 3.0, 3.0], np.float64)
    return m, s, 0.1


def _build_program(reps=1, cfg=None, params=None):
    """Build + compile the per-core Bass/Tile program (identical on all cores).

    reps>1 wraps the whole body in a device-side loop — used only by the
    timing harness to amplify kernel time over dispatch/transfer noise.
    params = (mean_row, std_row, weight0); the derived constants are baked
    into the instruction stream.
    """
    import concourse.bacc as bacc
    import concourse.bass as bass
    import concourse.mybir as mybir
    import concourse.tile as tile

    import bass_rust

    OP_VRECIP, OP_MULCLAMP, OP_SQMIN, OP_SQMINADD, OP_SCANMUL = _get_ops()

    cfg = dict(cfg or {})
    if params is None:
        m, sd, w0 = _default_params()
    else:
        m, sd, w0 = params
    m = np.asarray(m, np.float64)
    sd = np.asarray(sd, np.float64)
    f = 1.0 - np.tanh(-float(w0))
    var = sd * sd
    clip = (NEG_LOG_EPS - 0.5 * np.log(2.0 * np.pi * var)) * f     # clip_i'
    K = f / (2.0 * var)                                            # K_i'
    sK = np.sqrt(K)
    # lp0 = min((sK0*na - sK0*m0)^2, clip0)
    s0_, b0_ = float(sK[0]), float(-sK[0] * m[0])
    # ang1 = 90 - R2D*h1 ;  sK1*(ang1-m1) = b1 + s1*h1
    s1_, b1_ = float(-sK[1] * R2D), float(sK[1] * (90.0 - m[1]))
    # ang2 = 90 + R2D*h2
    s2_, b2_ = float(sK[2] * R2D), float(sK[2] * (90.0 - m[2]))
    c0_, c1_, c2_ = float(clip[0]), float(clip[1]), float(clip[2])

    W = cfg.get("W", 512)
    tiles = cfg.get("tiles")
    if tiles is None:
        if cfg.get("uneven", True) and W == 512 and RES_PER_PART == 2048:
            tiles = [256, 512, 512, 512, 256]
        else:
            tiles = [W] * (RES_PER_PART // W)
    assert sum(tiles) == RES_PER_PART or cfg.get("partial", False)
    offs = [0]
    for w_ in tiles:
        offs.append(offs[-1] + w_)
    NT = len(tiles)
    xbufs = cfg.get("xbufs", 2)
    midbufs = cfg.get("midbufs", 2)
    bigbufs = cfg.get("bigbufs", 2)
    ph2bufs = cfg.get("ph2bufs", 1)

    dt = mybir.dt
    Alu = mybir.AluOpType
    Act = mybir.ActivationFunctionType

    nc = bacc.Bacc(
        "TRN2",
        target_bir_lowering=False,
        debug=False,
        enable_asserts=False,
        num_devices=NCORES,
    )

    xin = nc.dram_tensor("xin", [(RES_PER_CORE + 1) * 9], dt.float32,
                         kind="ExternalInput")
    out_dt = dt.float16 if cfg.get("tail_eng", "dve16") == "dve16" else dt.float32
    out = nc.dram_tensor("out", [RES_PER_CORE], out_dt,
                         kind="ExternalOutput")

    # activation bias/scale floats resolve through the const-AP database;
    # register the baked constants (memset [P,1] tiles, same as bass init)
    for _v in {s0_, b0_, s1_, b1_, s2_, b2_}:
        if (dt.float32, _v) not in nc.const_aps.aps:
            _ct = nc.alloc_sbuf_tensor(
                f"blc-const-{len(nc.const_aps.aps)}", [P, 1], dt.float32)
            nc.gpsimd.memset(_ct.ap(), _v)
            nc.const_aps.aps[(dt.float32, _v)] = _ct.ap()
    nc.all_engine_barrier()

    def eng(name, default):
        e = cfg.get(name, default)
        return {"dve": nc.vector, "pool": nc.gpsimd}[e]

    with tile.TileContext(nc) as tc:
        with (
            tc.tile_pool(name="xpool", bufs=xbufs) as xpool,
            tc.tile_pool(name="dpool", bufs=bigbufs) as dpool,
            tc.tile_pool(name="spool", bufs=cfg.get("spool_bufs", 1)) as spool,
            tc.tile_pool(name="ppool", bufs=bigbufs) as ppool,
            tc.tile_pool(name="mid", bufs=midbufs) as mid,
            tc.tile_pool(name="sxp", bufs=cfg.get("sx_bufs", 1)) as sxp,
            tc.tile_pool(name="xph", bufs=NT) as xph,     # crosses phase bound
            tc.tile_pool(name="ph2", bufs=ph2bufs) as ph2,
        ):
            def _body():
                ratios = []
                accs = []
                ph1_act = []

                def emit_ph2(t, ratio, acc0):
                    Wt = tiles[t]
                    h = ph2.tile([P, 2 * Wt], dt.float32, tag="h")
                    h_inst = nc.scalar.activation(h[:, :], ratio[:, :],
                                                  Act.Arctan)
                    if cfg.get("fence", True):
                        for a in ph1_act:
                            bass_rust.add_dep_helper(
                                h_inst.ins, a.ins,
                                reason="sqrt-table set before arctan")
                    hv = h[:, :].rearrange("p (w t) -> p w t", t=2)
                    if cfg.get("contig", False):
                        h1in, h2in = h[:, 0:Wt], h[:, Wt:2 * Wt]
                    else:
                        h1in, h2in = hv[:, :, 1], hv[:, :, 0]
                    te = cfg.get("tail_eng", "dve16")
                    if te in ("pool", "dve16"):
                        # scaled squares on Act, min/add chain off the DVE
                        # critical path (Pool) or on DVE in fp16
                        tdt = dt.float16 if te == "dve16" else dt.float32
                        teng = nc.gpsimd if te == "pool" else nc.vector
                        sq1 = ph2.tile([P, Wt], tdt, tag="sq1")
                        nc.scalar.activation(sq1[:, :], h1in, Act.Square,
                                             bias=b1_, scale=s1_)
                        sq2 = ph2.tile([P, Wt], tdt, tag="sq2")
                        nc.scalar.activation(sq2[:, :], h2in, Act.Square,
                                             bias=b2_, scale=s2_)
                        t0 = ph2.tile([P, Wt], tdt, tag="t0")
                        teng.tensor_scalar(t0[:, :], acc0[:, :], c0_,
                                           None, op0=Alu.min)
                        acc1 = ph2.tile([P, Wt], tdt, tag="acc1")
                        teng.scalar_tensor_tensor(
                            acc1[:, :], sq1[:, :], c1_, t0[:, :],
                            op0=Alu.min, op1=Alu.add)
                        acc2 = ph2.tile([P, Wt], tdt, tag="acc2")
                        teng.scalar_tensor_tensor(
                            acc2[:, :], sq2[:, :], c2_, acc1[:, :],
                            op0=Alu.min, op1=Alu.add)
                    else:
                        acc1 = ph2.tile([P, Wt], dt.float32, tag="acc1")
                        nc.vector._custom_dve(OP_SQMINADD, out=acc1[:, :],
                                              in0=h1in, in1=acc0[:, :],
                                              s0=s1_, s1=b1_, imm2=c1_)
                        acc2 = ph2.tile([P, Wt], dt.float32, tag="acc2")
                        nc.vector._custom_dve(OP_SQMINADD, out=acc2[:, :],
                                              in0=h2in, in1=acc1[:, :],
                                              s0=s2_, s1=b2_, imm2=c2_)
                    dst = bass.AP(out, P * offs[t], [[Wt, P], [1, Wt]])
                    out_eng = {"sp": nc.sync, "act": nc.scalar,
                               "gpsimd": nc.gpsimd}[cfg.get("out_dma", "gpsimd")]
                    out_eng.dma_start(dst, acc2[:, :])

                # ---------------- phase 1 ------------------------------------
                for t in range(NT):
                    Wt = tiles[t]
                    FW = 9 * Wt
                    XW = 9 * (Wt + 1)
                    base = P * offs[t]
                    contig = cfg.get("contig", False)
                    x = xpool.tile([P, XW], dt.float32, tag="x")
                    xde = cfg.get("x_dma", "sp")
                    if cfg.get("compute_only", False):
                        # tiny DMA allocates the tile; compute reads mostly
                        # uninitialized SBUF (timing diagnostic only)
                        nc.sync.dma_start(
                            x[:, 0:2], bass.AP(xin, 0, [[2, P], [1, 2]]))
                        xde = "none"
                    if xde == "split":
                        # halves on different queue engines (SP + Act)
                        h1w = XW // 2
                        nc.sync.dma_start(
                            x[:, 0:h1w],
                            bass.AP(xin, base * 9, [[FW, P], [1, h1w]]))
                        nc.scalar.dma_start(
                            x[:, h1w:XW],
                            bass.AP(xin, base * 9 + h1w,
                                    [[FW, P], [1, XW - h1w]]))
                    elif xde == "alt":
                        e = nc.sync if t % 2 == 0 else nc.scalar
                        e.dma_start(x[:, :],
                                    bass.AP(xin, base * 9, [[FW, P], [1, XW]]))
                    elif xde == "split2":
                        h1w = XW // 2
                        nc.sync.dma_start(
                            x[:, 0:h1w],
                            bass.AP(xin, base * 9, [[FW, P], [1, h1w]]))
                        nc.sync.dma_start(
                            x[:, h1w:XW],
                            bass.AP(xin, base * 9 + h1w,
                                    [[FW, P], [1, XW - h1w]]))
                    elif xde != "none":
                        src = bass.AP(xin, base * 9, [[FW, P], [1, XW]])
                        nc.sync.dma_start(x[:, :], src)
                    if cfg.get("dma_only", False):
                        ratios.append(None)
                        accs.append(None)
                        continue

                    # D[i] = X[i+6] - X[i+3]; per bond j:
                    #   D[9j+0..2]=v_cac, D[9j+3..5]=v_cn, D[9j+6..8]=v_nca
                    d = dpool.tile([P, FW], dt.float32, tag="d")
                    de = eng("d_eng", "dve")
                    if cfg.get("d_split2", True):
                        hh = FW // 2
                        de.tensor_sub(d[:, 0:hh], x[:, 6:6 + hh],
                                      x[:, 3:3 + hh])
                        de.tensor_sub(d[:, hh:FW], x[:, 6 + hh:6 + FW],
                                      x[:, 3 + hh:3 + FW])
                    else:
                        de.tensor_sub(d[:, :], x[:, 6:6 + FW], x[:, 3:3 + FW])

                    # squares of all components (scalar engine)
                    s = spool.tile([P, FW], dt.float32, tag="s")
                    ph1_act.append(
                        nc.scalar.activation(s[:, :], d[:, :], Act.Square))

                    # P6[6j+m] = D[9j+m]*D[9j+m+3], m=0..5 (only for the
                    # non-scan dots path)
                    p6 = None
                    if not (cfg.get("scandots", False) and not contig):
                        d3 = d[:, :].rearrange("p (w k) -> p w k", k=9)
                        p6 = ppool.tile([P, 6 * Wt], dt.float32, tag="p6")
                        p6v = p6[:, :].rearrange("p (w k) -> p w k", k=6)
                        if contig:
                            eng("p6_eng", "dve").tensor_tensor(
                                p6[:, :], d[:, 0:6 * Wt],
                                d[:, 3 * Wt:9 * Wt], op=Alu.mult)
                        else:
                            eng("p6_eng", "dve").tensor_tensor(
                                p6v, d3[:, :, 0:6], d3[:, :, 3:9],
                                op=Alu.mult)

                    # windowed 3-sums of squares: R2[j] = (nc2, na2, nb2)
                    sv = s[:, :].rearrange("p (w t k) -> p w t k", t=3, k=3)
                    r2 = mid.tile([P, 3 * Wt], dt.float32, tag="r2")
                    r2v = r2[:, :].rearrange("p (w t) -> p w t", t=3)
                    r2e = eng("r2_eng", "dve")
                    if contig:
                        r2e.tensor_tensor(r2[:, :], s[:, 0:3 * Wt],
                                          s[:, 3 * Wt:6 * Wt], op=Alu.add)
                        r2e.tensor_tensor(r2[:, :], r2[:, :],
                                          s[:, 6 * Wt:9 * Wt], op=Alu.add)
                    else:
                        r2e.tensor_tensor(r2v, sv[:, :, :, 0], sv[:, :, :, 1],
                                          op=Alu.add)
                        r2e.tensor_tensor(r2v, r2v, sv[:, :, :, 2],
                                          op=Alu.add)
                    r2t = r2[:, :].rearrange("p (w t) -> p w t", t=3)

                    # dots: DOTS[j] = (dot2, dot1)
                    dots = mid.tile([P, 2 * Wt], dt.float32, tag="dots")
                    dotsv = dots[:, :].rearrange("p (w t) -> p w t", t=2)
                    if cfg.get("scandots", False) and not contig:
                        # dots from running sum of pair products:
                        #   S[i] = cumsum(d[g(i)]*d[g(i)+3]), i in group-of-6
                        #   dot2_j = S[6j+2]-S[6j-1]; dot1_j = S[6j+5]-S[6j+2]
                        dview = d[:, :].rearrange("p (w k) -> p w k", k=9)
                        sx = sxp.tile([P, 6 * Wt + 5], dt.float32, tag="sx")
                        nc.gpsimd.memset(sx[:, 2:3], 0.0)
                        nc.vector._custom_dve(
                            OP_SCANMUL, out=sx[:, 3:6 * Wt + 3],
                            in0=dview[:, :, 0:6], in1=dview[:, :, 3:9])
                        av = sx[:, 5:5 + 6 * Wt].rearrange(
                            "p (w a b) -> p w a b", a=2, b=3)
                        bv = sx[:, 2:2 + 6 * Wt].rearrange(
                            "p (w a b) -> p w a b", a=2, b=3)
                        nc.vector.tensor_tensor(dotsv, av[:, :, :, 0],
                                                bv[:, :, :, 0],
                                                op=Alu.subtract)
                    else:
                        pv = p6[:, :].rearrange("p (w t k) -> p w t k",
                                                t=2, k=3)
                        dte = eng("dots_eng", "dve")
                        if contig:
                            dte.tensor_tensor(dots[:, :], p6[:, 0:2 * Wt],
                                              p6[:, 2 * Wt:4 * Wt],
                                              op=Alu.add)
                            dte.tensor_tensor(dots[:, :], dots[:, :],
                                              p6[:, 4 * Wt:6 * Wt],
                                              op=Alu.add)
                        else:
                            dte.tensor_tensor(dotsv, pv[:, :, :, 0],
                                              pv[:, :, :, 1], op=Alu.add)
                            dte.tensor_tensor(dotsv, dotsv, pv[:, :, :, 2],
                                              op=Alu.add)

                    # q interleaved to match DOTS: (q2, q1) = (nc2*na2, na2*nb2)
                    q = mid.tile([P, 2 * Wt], dt.float32, tag="q")
                    qv = q[:, :].rearrange("p (w t) -> p w t", t=2)
                    if contig:
                        eng("q_eng", "dve").tensor_tensor(
                            q[:, :], r2[:, 0:2 * Wt], r2[:, Wt:3 * Wt],
                            op=Alu.mult)
                    else:
                        eng("q_eng", "dve").tensor_tensor(
                            qv, r2t[:, :, 0:2], r2t[:, :, 1:3], op=Alu.mult)

                    # rv1 ~= 1/max(q - dots^2, tiny)
                    rv1 = mid.tile([P, 2 * Wt], dt.float32, tag="rv1")
                    nc.vector._custom_dve(OP_VRECIP, out=rv1[:, :],
                                          in0=dots[:, :], in1=q[:, :],
                                          s0=TINY, s1=RCP_C1, imm2=RCP_C2)
                    # srv = sqrt(rv1) = 1/sqrt(v)
                    srv = mid.tile([P, 2 * Wt], dt.float32, tag="srv")
                    ph1_act.append(
                        nc.scalar.activation(srv[:, :], rv1[:, :], Act.Sqrt))
                    # ratio = clamp(dots * srv) to the arctan domain
                    ratio = xph.tile([P, 2 * Wt], dt.float32, tag="ratio")
                    nc.vector._custom_dve(OP_MULCLAMP, out=ratio[:, :],
                                          in0=dots[:, :], in1=srv[:, :],
                                          s0=1.5707, s1=-1.5707)

                    # bond length: na = sqrt(na2); acc0 = min((s0*na+b0)^2, c0)
                    na = mid.tile([P, Wt], dt.float32, tag="na")
                    na_in = r2[:, 0:Wt] if contig else r2t[:, :, 1]
                    ph1_act.append(
                        nc.scalar.activation(na[:, :], na_in, Act.Sqrt))
                    te_ = cfg.get("tail_eng", "dve16")
                    if te_ in ("pool", "dve16"):
                        # unclamped (s0*na+b0)^2; min applied in ph2
                        a0dt = dt.float16 if te_ == "dve16" else dt.float32
                        acc0 = xph.tile([P, Wt], a0dt, tag="acc0")
                        ph1_act.append(
                            nc.scalar.activation(acc0[:, :], na[:, :],
                                                 Act.Square, bias=b0_,
                                                 scale=s0_))
                    else:
                        acc0 = xph.tile([P, Wt], dt.float32, tag="acc0")
                        nc.vector._custom_dve(OP_SQMIN, out=acc0[:, :],
                                              in0=na[:, :], s0=s0_, s1=b0_,
                                              imm2=c0_)

                    ratios.append(ratio)
                    accs.append(acc0)
                    if cfg.get("interleave", False):
                        emit_ph2(t, ratio, acc0)

                if not cfg.get("interleave", False) and not cfg.get("dma_only", False):
                    for t in range(NT):
                        emit_ph2(t, ratios[t], accs[t])

            if reps == 1:
                _body()
            else:
                with tc.For_i(0, reps, 1):
                    _body()

    nc.compile()
    return nc


def _get_program(params=None):
    if params is None:
        m, sd, w0 = _default_params()
    else:
        m, sd, w0 = params
    key = (tuple(np.asarray(m, np.float64)),
           tuple(np.asarray(sd, np.float64)), float(w0))
    prog = _PROGRAMS.get(key)
    if prog is None:
        prog = _build_program(params=(m, sd, w0))
        _PROGRAMS[key] = prog
    return prog


# ---------------------------------------------------------------- host side
def _is_canonical(ad, coords, mean, std):
    if ad.shape != (N_ATOMS, 5) or coords.shape != (N_ATOMS, 3):
        return False
    if mean.shape != (20, 3) or std.shape != (20, 3):
        return False
    if not (np.all(mean == mean[0:1]) and np.all(std == std[0:1])):
        return False
    if not np.all(std[0] > 0):
        return False
    a5 = ad.reshape(B, NCH, NRES, APR, 5)
    if not np.all(a5[..., 0] == np.arange(B, dtype=ad.dtype)[:, None, None, None]):
        return False
    if not np.all(a5[..., 1] == np.arange(NCH, dtype=ad.dtype)[:, None, None]):
        return False
    if not np.all(a5[..., 2] == np.arange(NRES, dtype=ad.dtype)[:, None]):
        return False
    if not np.all(a5[..., 4] == np.arange(APR, dtype=ad.dtype)):
        return False
    if not np.isfinite(coords).all() or np.abs(coords).max() >= 1e4:
        return False
    # all bond-geometry norms must clear the reference's EPS mask, so the
    # device kernel can skip mask arithmetic entirely
    r = coords.reshape(B, NCH, NRES, 9)
    w = r[:, :, 1:, 0:3] - r[:, :, :-1, 6:9]
    a = r[:, :, 1:, 3:6] - r[:, :, 1:, 0:3]
    bb = r[:, :, :-1, 6:9] - r[:, :, :-1, 3:6]
    mn = min(
        (w * w).sum(-1).min(),
        (a * a).sum(-1).min(),
        (bb * bb).sum(-1).min(),
    )
    return bool(mn > 1.1e-16)


def _run_fast(coords, mean, std, weight, n_alt):
    from concourse import bass_utils

    nc = _get_program(params=(mean[0].astype(np.float64),
                              std[0].astype(np.float64), float(weight[0])))
    cflat = np.ascontiguousarray(coords.reshape(-1), dtype=np.float32)
    in_maps = []
    for c in range(NCORES):
        shard = np.empty(((RES_PER_CORE + 1) * 9,), dtype=np.float32)
        shard[:-9] = cflat[c * ATOMS_PER_CORE * 3:(c + 1) * ATOMS_PER_CORE * 3]
        shard[-9:] = _PAD_RESIDUE
        in_maps.append({"xin": shard})

    res = bass_utils.run_bass_kernel_spmd(nc, in_maps,
                                          core_ids=list(range(NCORES)))
    parts = [np.asarray(res.results[c]["out"]).astype(np.float32)
             for c in range(NCORES)]
    e = np.concatenate(parts).reshape(B, NCH, NRES)
    e[:, :, NRES - 1] = 0.0          # no bond out of the last residue
    full = np.zeros((B, NCH, NRES, n_alt), dtype=np.float32)
    full[..., 0] = e
    return full


# ------------------------------------------------------------ numpy fallback
def _fallback(ad, coords, alternatives, weight, mean, std):
    """Faithful numpy port of the jax reference (incl. OOB drop/clamp)."""
    n_alt = alternatives.shape[-1]
    batch, chain, resnum = ad[:, 0], ad[:, 1], ad[:, 2]
    resname, at_name = ad[:, 3], ad[:, 4]
    n = ad.shape[0]

    table = np.full((B, NCH, NRES, APR), -1, dtype=np.int32)
    ok = ((batch >= 0) & (batch < B) & (chain >= 0) & (chain < NCH)
          & (resnum >= 0) & (resnum < NRES) & (at_name >= 0) & (at_name < APR))
    idx = np.arange(n, dtype=np.int32)
    table[batch[ok], chain[ok], resnum[ok], at_name[ok]] = idx[ok]

    c_idx = table[:, :, :-1, 2].reshape(-1)
    n_idx = table[:, :, 1:, 0].reshape(-1)
    cac_idx = table[:, :, :-1, 1].reshape(-1)
    can_idx = table[:, :, 1:, 1].reshape(-1)
    valid_idx = (c_idx >= 0) & (n_idx >= 0) & (cac_idx >= 0) & (can_idx >= 0)
    safe = lambda i: np.where(i >= 0, i, 0)

    co = coords.astype(np.float32)
    c_xyz = co[safe(c_idx)]
    n_xyz = co[safe(n_idx)]
    cac_xyz = co[safe(cac_idx)]
    can_xyz = co[safe(can_idx)]

    v_cn = n_xyz - c_xyz
    v_nca = can_xyz - n_xyz
    v_cac = c_xyz - cac_xyz

    def ang_deg(a, b):
        na = np.sqrt((a * a).sum(-1))
        nb = np.sqrt((b * b).sum(-1))
        mask = (na > EPS) & (nb > EPS)
        cos = np.clip((a * b).sum(-1) / (na * nb + EPS), -1.0, 1.0)
        return np.degrees(np.arccos(cos)).astype(np.float32), mask

    ang1, m1 = ang_deg(v_cn, v_nca)
    ang2, m2 = ang_deg(v_cac, -v_cn)
    bond_len = np.sqrt((v_cn * v_cn).sum(-1))
    valid = valid_idx & m1 & m2

    geom = np.stack([bond_len, ang1, ang2], axis=-1)
    seq = np.clip(resname[safe(c_idx)], 0, 19)
    var = (std.astype(np.float32)[seq]) ** 2
    denom = np.sqrt(2.0 * np.pi * var).astype(np.float32)
    num = np.exp(-((geom - mean.astype(np.float32)[seq]) ** 2) / (2.0 * var))
    log_prob = -(np.log(np.clip(num / denom, EPS, None)) + np.log(denom))
    scores = log_prob.sum(-1)

    f = np.float32(1.0 - np.tanh(-np.float32(weight[0])))
    val = np.where(valid, scores * f, 0.0).astype(np.float32)

    b_c = batch[safe(c_idx)]
    ch_c = chain[safe(c_idx)]
    r_c = resnum[safe(c_idx)]
    resi = np.zeros((B, NCH, NRES, n_alt), dtype=np.float32)
    ok2 = ((b_c >= 0) & (b_c < B) & (ch_c >= 0) & (ch_c < NCH)
           & (r_c >= 0) & (r_c < NRES))
    resi[b_c[ok2], ch_c[ok2], r_c[ok2], 0] = val[ok2]
    return resi


# ----------------------------------------------------------------- entry
def kernel(atom_description, coords, alternatives, weight, mean, std):
    ad = np.asarray(atom_description)
    co = np.asarray(coords, dtype=np.float32)
    al = np.asarray(alternatives)
    wt = np.asarray(weight, dtype=np.float32)
    mn = np.asarray(mean, dtype=np.float32)
    sd = np.asarray(std, dtype=np.float32)

    if _is_canonical(ad, co, mn, sd):
        return _run_fast(co, mn, sd, wt, al.shape[-1])
    return _fallback(ad, co, al, wt, mn, sd)
